# revision 12
# baseline (speedup 1.0000x reference)
"""Causal MoE (top-1) Trainium2 kernel.

Strategy (expert-parallel, 8 cores):
  Phase 1 (device): per-batch causal routing. logits = cumsum(x @ router_w)/t,
    computed with triangular-mask matmuls; batched softmax max-prob gate +
    argmax idx over all 16 token chunks at once.
  Host: group token ids by expert id; build a segmented SPMD plan: every core
    processes the same fixed segment sizes (sum C ~ 1024+pad), each (core,
    segment) slot bound to one expert via its per-core weight inputs. This
    keeps all 8 cores load-balanced even when expert counts are skewed.
  Phase 2 (device): per-core segmented dense FFN:
    ygT = gate * (gelu_tanh(x @ w1 + b1) @ w2 + b2), all feature-major.
    Weights move in [128, 1024] blocks (few, large DMAs); w2 for the running
    segment stays SBUF-resident so the second token chunk re-uses it.
  Host: scatter rows back to [B, S, D].
"""

import os

import ml_dtypes
import numpy as np
from contextlib import ExitStack

import concourse.bass as bass
import concourse.mybir as mybir
import concourse.tile as tile
from concourse.bass_utils import run_bass_kernel_spmd
from concourse.vector_clock import ScopedClock


def _run_spmd(nc, in_maps, core_ids, retries=2):
    """run_bass_kernel_spmd with retry on transient device faults."""
    last = None
    for attempt in range(retries + 1):
        try:
            return run_bass_kernel_spmd(nc, in_maps, core_ids=core_ids)
        except Exception as e:  # e.g. NRT_EXEC_UNIT_UNRECOVERABLE blips
            last = e
            if attempt == retries:
                raise
    raise last

B, S, D, F, E = 4, 2048, 1024, 4096, 4
NCORES = 8
NCH = S // 128  # 16 token chunks per batch
ND = D // 128  # 8
NF = F // 128  # 32
NFB = NF // 8  # 4 weight blocks of 8 f-tiles
FP = mybir.dt.float32
AFT = mybir.ActivationFunctionType
AX = mybir.AxisListType
ALU = mybir.AluOpType

# ---------------------------------------------------------------------------
# Workaround: the walrus build here allows only 1 sync-wait per instruction
# (setupSyncWait "Too many sync wait commands"), while Tile may attach
# several. Split the extra waits onto carrier NoOps on the same engine,
# executed immediately before the instruction (same stream => same
# semantics). Applied both to the scheduled instruction lists and to the
# TileContext tail drain.
_PATCHED = False

_orig_lower_ordered_insts = tile.TileContext._lower_ordered_insts


def _split_waits_lower(self, ordered):
    nc = self.nc
    for insts in ordered.values():
        new = []
        for inst in insts:
            si = getattr(inst, "sync_info", None)
            eng = getattr(inst, "engine", None)
            if (
                si is not None
                and si.on_wait
                and len(si.on_wait) > 1
                and eng is not None
                and eng != mybir.EngineType.Unassigned
            ):
                waits = list(si.on_wait)
                si.on_wait.clear()
                si.on_wait.append(waits[-1])
                for w in waits[:-1]:
                    nop = mybir.InstNoOp(
                        name=nc.get_next_instruction_name(), ins=[], outs=[]
                    )
                    nop.engine = eng
                    nop.sync_info = mybir.SyncInfo(on_wait=[w], on_update=[])
                    new.append(nop)
            new.append(inst)
        insts[:] = new
    return _orig_lower_ordered_insts(self, ordered)


def _patched_drain_and_barrier(self, tick_clock, wait_clock):
    nc = self.nc
    drain_inst = nc.sync.drain()
    wait_clock.add_sem_waits(
        drain_inst.ins, ScopedClock({None: tick_clock.global_clock})
    )
    si = drain_inst.ins.sync_info
    if si is not None and len(si.on_wait) > 1:
        waits = list(si.on_wait)
        si.on_wait.clear()
        si.on_wait.append(waits[0])
        for w in waits[1:]:
            nop = nc.sync.nop(nofuse=True, hint="drain_wait_spill")
            nsi = nop.ins.sync_info
            if nsi is None:
                nop.ins.sync_info = mybir.SyncInfo(on_wait=[], on_update=[])
                nsi = nop.ins.sync_info
            nsi.on_wait.append(w)
    nc.all_engine_barrier()
    assert self.sems is not None
    popped = nc._tile_sem_poison_stack.pop()
    assert popped is self._sem_poison
    nc.clear_and_free_semaphores(list(self.sems.allocated().values()))
    nc.all_engine_barrier()


def _patch_tile_drain():
    global _PATCHED
    if not _PATCHED:
        tile.TileContext._drain_and_barrier = _patched_drain_and_barrier
        tile.TileContext._lower_ordered_insts = _split_waits_lower
        _PATCHED = True


# ---------------------------------------------------------------------------
# Phase 1: routing. One batch (2048 tokens) per core.


def build_phase1(hw_loop=0):
    _patch_tile_drain()
    nc = bass.Bass("TRN2", num_devices=NCORES, debug=False)
    xbT = nc.dram_tensor("xbT", [D, S], FP, kind="ExternalInput")
    rw = nc.dram_tensor("rw", [D, E], FP, kind="ExternalInput")
    ubig = nc.dram_tensor("ubig", [128, S], FP, kind="ExternalInput")
    masks = nc.dram_tensor("masks", [NCH, S], FP, kind="ExternalInput")
    ind = nc.dram_tensor("ind", [S, NCH], FP, kind="ExternalInput")
    iotar = nc.dram_tensor("iotar", [128, E * NCH], FP, kind="ExternalInput")
    idx_out = nc.dram_tensor("idx_out", [128, NCH], FP, kind="ExternalOutput")
    gate_out = nc.dram_tensor("gate_out", [128, NCH], FP, kind="ExternalOutput")

    with tile.TileContext(nc) as tc, ExitStack() as ctx:
        cpool = ctx.enter_context(tc.tile_pool(name="const", bufs=1))
        ppool = ctx.enter_context(tc.tile_pool(name="persist", bufs=1))
        wpool = ctx.enter_context(tc.tile_pool(name="work", bufs=2))
        pspool = ctx.enter_context(tc.tile_pool(name="psum", bufs=1, space="PSUM"))

        loop_ctx = tc.For_i(0, hw_loop, 1) if hw_loop else None
        if loop_ctx is not None:
            loop_ctx.__enter__()

        xbT_t = []
        for d in range(8):
            t = cpool.tile([128, S], FP, name=f"xbT{d}", tag=f"xbT{d}")
            # alternate DMA queues to engage more DMA engines in parallel
            eng = nc.sync if d % 2 == 0 else nc.scalar
            eng.dma_start(t[:], xbT.ap()[d * 128 : (d + 1) * 128, :])
            xbT_t.append(t)
        rw_t = []
        for d in range(8):
            t = cpool.tile([128, E], FP, name=f"rw{d}", tag=f"rw{d}")
            nc.sync.dma_start(t[:], rw.ap()[d * 128 : (d + 1) * 128, :])
            rw_t.append(t)
        ubig_t = cpool.tile([128, S], FP, name="ubig", tag="ubig")
        nc.scalar.dma_start(ubig_t[:], ubig.ap()[:])
        masks_t = cpool.tile([NCH, S], FP, name="masks", tag="masks")
        nc.scalar.dma_start(masks_t[:], masks.ap()[:])
        ind_t = []
        for c in range(NCH):
            t = cpool.tile([128, NCH], FP, name=f"ind{c}", tag=f"ind{c}")
            nc.scalar.dma_start(t[:], ind.ap()[c * 128 : (c + 1) * 128, :])
            ind_t.append(t)
        iotar_t = cpool.tile([128, E * NCH], FP, name="iotar", tag="iotar")
        nc.scalar.dma_start(iotar_t[:], iotar.ap()[:])

        # r[t, e] per chunk c at columns [E*c, E*(c+1)). One single-d pass per
        # ping-pong PSUM bank (16 one-shot groups each; a PSUM bank allows
        # only one OPEN accumulation group at a time), accumulated into SBUF
        # by DVE adds — so each d-tile's matmuls chase its own DMA.
        r_t = ppool.tile([128, E * NCH], FP, name="r", tag="r")
        for d in range(8):
            ps_r = pspool.tile(
                [128, E * NCH], FP, name=f"ps_r{d % 2}", tag=f"ps_r{d % 2}"
            )
            for c in range(NCH):
                nc.tensor.matmul(
                    ps_r[:, c * E : (c + 1) * E],
                    xbT_t[d][:, c * 128 : (c + 1) * 128],
                    rw_t[d][:],
                    start=True,
                    stop=True,
                )
            if d == 0:
                nc.vector.tensor_copy(r_t[:], ps_r[:])
            else:
                nc.vector.tensor_add(r_t[:], r_t[:], ps_r[:])

        # S[c, e] = sum over chunk c of r
        ps_S = pspool.tile([NCH, E], FP, name="ps_S", tag="ps_S")
        for c in range(NCH):
            nc.tensor.matmul(
                ps_S[:],
                ind_t[c][:],
                r_t[:, c * E : (c + 1) * E],
                start=(c == 0),
                stop=(c == NCH - 1),
            )
        S_t = ppool.tile([NCH, E], FP, name="S", tag="S")
        nc.vector.tensor_copy(S_t[:], ps_S[:])

        # logits (pre-scaled by 1/(t+1) via the mask constants), all chunks
        # into one PSUM bank [128, NCH*E]
        lg = pspool.tile([128, E * NCH], FP, name="ps_lg", tag="ps_lg")
        for c in range(NCH):
            nc.tensor.matmul(
                lg[:, c * E : (c + 1) * E],
                ubig_t[:, c * 128 : (c + 1) * 128],
                r_t[:, c * E : (c + 1) * E],
                start=True,
                stop=False,
                skip_group_check=True,
            )
            nc.tensor.matmul(
                lg[:, c * E : (c + 1) * E],
                masks_t[:, c * 128 : (c + 1) * 128],
                S_t[:],
                start=False,
                stop=True,
                skip_group_check=True,
            )

        # batched softmax + argmax over all chunks: APs viewed [128, NCH, E]
        lg3 = lg[:].rearrange("p (c e) -> p c e", c=NCH)
        mx = wpool.tile([128, NCH], FP, name="mx", tag="mx")
        nc.vector.reduce_max(mx[:], lg3, axis=AX.X)
        mx3 = mx[:].unsqueeze(2).broadcast_to([128, NCH, E])
        dif = wpool.tile([128, E * NCH], FP, name="dif", tag="dif")
        dif3 = dif[:].rearrange("p (c e) -> p c e", c=NCH)
        nc.vector.tensor_sub(dif3, lg3, mx3)
        et = wpool.tile([128, E * NCH], FP, name="et", tag="et")
        nc.scalar.activation(et[:], dif[:], AFT.Exp)
        et3 = et[:].rearrange("p (c e) -> p c e", c=NCH)
        ssum = wpool.tile([128, NCH], FP, name="ssum", tag="ssum")
        nc.vector.reduce_sum(ssum[:], et3, axis=AX.X)
        gate_t = ppool.tile([128, NCH], FP, name="gate", tag="gate")
        nc.vector.reciprocal(gate_t[:], ssum[:])
        msk = wpool.tile([128, E * NCH], FP, name="msk", tag="msk")
        msk3 = msk[:].rearrange("p (c e) -> p c e", c=NCH)
        nc.vector.tensor_tensor(msk3, lg3, mx3, op=ALU.is_equal)
        nc.vector.tensor_mul(msk[:], msk[:], iotar_t[:])
        rev = wpool.tile([128, NCH], FP, name="rev", tag="rev")
        nc.vector.reduce_max(rev[:], msk3, axis=AX.X)
        idx_t = ppool.tile([128, NCH], FP, name="idx", tag="idx")
        # idx = 3 - rev  (picks FIRST max on ties, like jnp.argmax)
        nc.scalar.activation(idx_t[:], rev[:], AFT.Copy, bias=3.0, scale=-1.0)

        nc.sync.dma_start(idx_out.ap()[:], idx_t[:])
        nc.sync.dma_start(gate_out.ap()[:], gate_t[:])
        if loop_ctx is not None:
            loop_ctx.__exit__(None, None, None)
    return nc


def phase1_constants():
    t = np.arange(S)
    p = t % 128
    c = t // 128
    inv = (1.0 / (t + 1.0)).astype(np.float32)
    q = np.arange(128)
    ubig = (q[:, None] <= p[None, :]).astype(np.float32) * inv[None, :]
    cp = np.arange(NCH)
    masks = (cp[:, None] < c[None, :]).astype(np.float32) * inv[None, :]
    ind = (c[:, None] == cp[None, :]).astype(np.float32)
    iota = np.tile(3.0 - np.arange(E, dtype=np.float32), NCH)
    iotar = np.broadcast_to(iota[None, :], (128, E * NCH)).copy()
    return {
        "ubig": np.ascontiguousarray(ubig),
        "masks": np.ascontiguousarray(masks),
        "ind": np.ascontiguousarray(ind),
        "iotar": iotar,
    }


def phase1_inmaps(x, router_w):
    consts = phase1_constants()
    in_maps = []
    for core in range(NCORES):
        b = core % B
        in_maps.append(
            {
                "xbT": np.ascontiguousarray(x[b].T),
                "rw": np.ascontiguousarray(router_w),
                **consts,
            }
        )
    return in_maps


def run_phase1(x, router_w):
    nc = build_phase1()
    res = _run_spmd(nc, phase1_inmaps(x, router_w), list(range(NCORES)))
    idx = np.empty((B, S), np.int32)
    gate = np.empty((B, S), np.float32)
    for b in range(B):
        # [128, 16] -> token t = c*128 + p
        idx[b] = (
            np.rint(res.results[b]["idx_out"]).astype(np.int32).T.reshape(S)
        )
        gate[b] = res.results[b]["gate_out"].T.reshape(S).astype(np.float32)
    return idx, gate


# ---------------------------------------------------------------------------
# Phase 2: segmented per-core expert FFN.
#
# All cores run the same program over `sizes` segments (sum = C tokens per
# core). Each (core, segment) slot is bound to one expert through its weight
# inputs, so skewed expert counts still give every core exactly C tokens.


def plan_segments(counts):
    """Pick segment sizes + (core, segment)->expert assignment minimizing C.

    Returns (sizes, assign) with assign[core][j] = expert for segment j.
    """
    counts = [int(c) for c in counts]

    def ceil32(v):
        return max(32, ((v + 31) // 32) * 32)

    # k=1 fallback: proportional greedy core allocation.
    alloc = [1] * E
    for _ in range(NCORES - E):
        loads = [counts[e] / alloc[e] for e in range(E)]
        alloc[int(np.argmax(loads))] += 1
    c1 = ceil32(max(-(-counts[e] // alloc[e]) for e in range(E)))
    best = (c1, [c1], [[e] for e in range(E) for _ in range(alloc[e])])

    # k=2 search: sizes (sA >= sB), both 32-aligned.
    for sA in range(128, 1025, 32):
        for sB in range(32, sA + 1, 32):
            C = sA + sB
            if C >= best[0] or NCORES * C < sum(counts):
                continue
            # enumerate A-segment compositions over experts
            found = None
            for a0 in range(NCORES + 1):
                for a1 in range(NCORES + 1 - a0):
                    for a2 in range(NCORES + 1 - a0 - a1):
                        a3 = NCORES - a0 - a1 - a2
                        a = (a0, a1, a2, a3)
                        need_b = [
                            max(0, -(-(counts[e] - a[e] * sA) // sB))
                            for e in range(E)
                        ]
                        if sum(need_b) <= NCORES:
                            # pad spare B slots onto expert 0
                            bslots = list(need_b)
                            bslots[0] += NCORES - sum(need_b)
                            found = (a, bslots)
                            break
                    if found:
                        break
                if found:
                    break
            if found:
                a, bslots = found
                a_list = [e for e in range(E) for _ in range(a[e])]
                b_list = [e for e in range(E) for _ in range(bslots[e])]
                assign = [[a_list[i], b_list[i]] for i in range(NCORES)]
                best = (C, [sA, sB], assign)
    return best[1], best[2]


def distribute_tokens(flat_idx, sizes, assign):
    """Fill (core, segment) slots with token ids of the slot's expert.

    Returns slot_ids[core][j] = np.array of token ids (len <= sizes[j]).
    """
    by_expert = {e: np.nonzero(flat_idx == e)[0] for e in range(E)}
    pos = {e: 0 for e in range(E)}
    slot_ids = [[None] * len(sizes) for _ in range(NCORES)]
    # fill large segments first so leftovers land in small ones
    order = sorted(
        [(core, j) for core in range(NCORES) for j in range(len(sizes))],
        key=lambda cj: -sizes[cj[1]],
    )
    for core, j in order:
        e = assign[core][j]
        ids = by_expert[e]
        take = min(sizes[j], len(ids) - pos[e])
        slot_ids[core][j] = np.asarray(ids[pos[e] : pos[e] + take], np.int64)
        pos[e] += take
    for e in range(E):
        assert pos[e] == len(by_expert[e]), "segment plan under-capacity"
    return slot_ids


def pack_w1(w1e):
    """[D, F] -> [NFB, ND, 128, 1024]: block (wb, d) holds 8 f-tiles of
    d-tile d, each DMA a contiguous [128, 1024] burst."""
    return np.ascontiguousarray(
        w1e.reshape(ND, 128, NFB, 1024).transpose(2, 0, 1, 3)
    )


def pack_w2(w2e):
    """[F, D] -> [NF, 128, D]: block fi is f-tile fi against all of D."""
    return np.ascontiguousarray(w2e.reshape(NF, 128, D))


def build_phase2_seg(sizes, bf16=True, hw_loop=0, repeat=1):
    """Segmented per-core FFN; single full-F pass per segment.

    Per segment: L1 computes all 32 f-tiles of h = gelu(x@w1+b1) (h resident
    in SBUF, bf16, tags shared across segments), then L2 accumulates
    y = h@w2 over all 32 f-tiles per token chunk with the segment's whole w2
    SBUF-resident, finalizing (y + b2) * gate in one DVE op per d-tile.
    Weight DMAs are [128, 1024] blocks (2 KB/partition bursts) to keep the
    global HWDGE dispatch (~625 ns per DMA) off the critical path. PSUM: 8
    banks; L1 ping-pongs two 4-bank sets across groups, L2 uses all 8.
    """
    _patch_tile_drain()
    nc = bass.Bass("TRN2", num_devices=NCORES, debug=False)
    C = sum(sizes)
    smax = max(sizes)
    WP = mybir.dt.bfloat16 if bf16 else FP
    NSEG = len(sizes)
    xgT = nc.dram_tensor("xgT", [D, C], WP, kind="ExternalInput")
    gateb = nc.dram_tensor("gateb", [128, C], FP, kind="ExternalInput")
    w1p = [
        nc.dram_tensor(f"w1p{j}", [NFB, ND, 128, 1024], WP, kind="ExternalInput")
        for j in range(NSEG)
    ]
    b1r = [
        nc.dram_tensor(f"b1r{j}", [128, NF], FP, kind="ExternalInput")
        for j in range(NSEG)
    ]
    w2p = [
        nc.dram_tensor(f"w2p{j}", [NF, 128, D], WP, kind="ExternalInput")
        for j in range(NSEG)
    ]
    b2r = [
        nc.dram_tensor(f"b2r{j}", [128, ND], FP, kind="ExternalInput")
        for j in range(NSEG)
    ]
    ygT = nc.dram_tensor("ygT", [D, C], FP, kind="ExternalOutput")

    with tile.TileContext(nc) as tc, ExitStack() as ctx:
        cpool = ctx.enter_context(tc.tile_pool(name="const", bufs=1))
        hpool = ctx.enter_context(tc.tile_pool(name="h", bufs=1))
        w1pool = ctx.enter_context(tc.tile_pool(name="w1", bufs=2))
        w2pool = ctx.enter_context(tc.tile_pool(name="w2", bufs=1))
        ypool = ctx.enter_context(tc.tile_pool(name="y", bufs=4))
        pspool = ctx.enter_context(tc.tile_pool(name="psum", bufs=1, space="PSUM"))

        xg_t = []
        for d in range(ND):
            t = cpool.tile([128, C], WP, name=f"xg{d}", tag=f"xg{d}")
            nc.sync.dma_start(t[:], xgT.ap()[d * 128 : (d + 1) * 128, :])
            xg_t.append(t)
        gate_t = cpool.tile([128, C], FP, name="gate", tag="gate")
        nc.scalar.dma_start(gate_t[:], gateb.ap()[:])
        b1_t = []
        b2_t = []
        for j in range(NSEG):
            t1 = cpool.tile([128, NF], FP, name=f"b1_{j}", tag=f"b1_{j}")
            nc.scalar.dma_start(t1[:], b1r[j].ap()[:])
            b1_t.append(t1)
            t2 = cpool.tile([128, ND], FP, name=f"b2_{j}", tag=f"b2_{j}")
            nc.scalar.dma_start(t2[:], b2r[j].ap()[:])
            b2_t.append(t2)

        h_t = [
            hpool.tile([128, smax], WP, name=f"h{fl}", tag=f"h{fl}")
            for fl in range(NF)
        ]

        loop_ctx = tc.For_i(0, hw_loop, 1) if hw_loop else None
        if loop_ctx is not None:
            loop_ctx.__enter__()
        for _rep in range(repeat):
          o_base = 0
          for j, sj in enumerate(sizes):
            ch = [(o, min(512, sj - o)) for o in range(0, sj, 512)]
            nch = len(ch)
            assert nch <= 2, f"segment size {sj} too large"
            # L1: h = gelu(x @ w1 + b1); groups of 2 f-tiles, two 4-bank sets
            w2_t = [None] * NF
            gsz = 2 if nch <= 2 else 1
            group_id = 0
            for wb in range(NFB):
                w1_t = []
                for d in range(ND):
                    t = w1pool.tile([128, 1024], WP, name=f"w1b{d}", tag=f"w1b{d}")
                    nc.sync.dma_start(t[:], w1p[j].ap()[wb, d])
                    w1_t.append(t)
                for sg in range(8 // gsz):
                    nbank = gsz * nch
                    base = 0 if (nbank > 4 or group_id % 2 == 0) else 4
                    pss = [
                        [
                            pspool.tile(
                                [128, 512], FP,
                                name=f"bk{base + fl * nch + q}",
                                tag=f"bk{base + fl * nch + q}",
                            )
                            for q in range(nch)
                        ]
                        for fl in range(gsz)
                    ]
                    group_id += 1
                    for d in range(ND):
                        for fl in range(gsz):
                            fcol = (sg * gsz + fl) * 128
                            for q, (o, n) in enumerate(ch):
                                nc.tensor.matmul(
                                    pss[fl][q][:, :n],
                                    w1_t[d][:, fcol : fcol + 128],
                                    xg_t[d][:, o_base + o : o_base + o + n],
                                    start=(d == 0),
                                    stop=(d == ND - 1),
                                )
                    for fl in range(gsz):
                        fi = wb * 8 + sg * gsz + fl
                        for q, (o, n) in enumerate(ch):
                            nc.scalar.activation(
                                h_t[fi][:, o : o + n],
                                pss[fl][q][:, :n],
                                AFT.Gelu_apprx_tanh,
                                bias=b1_t[j][:, fi : fi + 1],
                                scale=1.0,
                            )
                # prefetch this wb's 8 w2 blocks on the scalar queue, emitted
                # after the gelus so they never delay a PSUM drain; whole w2
                # is SBUF-resident by the time L2 starts
                for fi in range(wb * 8, wb * 8 + 8):
                    t = w2pool.tile([128, D], WP, name=f"w2b{fi}", tag=f"w2b{fi}")
                    nc.scalar.dma_start(t[:], w2p[j].ap()[fi])
                    w2_t[fi] = t
            # L2: y = (h @ w2 + b2) * gate; per chunk, all 8 d-banks at once
            for q, (o, n) in enumerate(ch):
                qss = [
                    pspool.tile([128, 512], FP, name=f"bk{dd}", tag=f"bk{dd}")
                    for dd in range(ND)
                ]
                for fi in range(NF):
                    for dd in range(ND):
                        nc.tensor.matmul(
                            qss[dd][:, :n],
                            w2_t[fi][:, dd * 128 : (dd + 1) * 128],
                            h_t[fi][:, o : o + n],
                            start=(fi == 0),
                            stop=(fi == NF - 1),
                        )
                for dd in range(ND):
                    yt = ypool.tile([128, 512], FP, name="yt", tag="yt")
                    nc.vector.scalar_tensor_tensor(
                        yt[:, :n],
                        qss[dd][:, :n],
                        b2_t[j][:, dd : dd + 1],
                        gate_t[:, o_base + o : o_base + o + n],
                        op0=ALU.add,
                        op1=ALU.mult,
                    )
                    nc.scalar.dma_start(
                        ygT.ap()[
                            dd * 128 : (dd + 1) * 128,
                            o_base + o : o_base + o + n,
                        ],
                        yt[:, :n],
                    )
            o_base += sj
        if loop_ctx is not None:
            loop_ctx.__exit__(None, None, None)
    return nc


DEFAULT_PREC = os.environ.get("MOE_PREC", "bf16")


def phase2_plan(idx):
    flat_idx = idx.reshape(-1)
    counts = np.bincount(flat_idx, minlength=E)
    sizes, assign = plan_segments(counts)
    slot_ids = distribute_tokens(flat_idx, sizes, assign)
    return sizes, assign, slot_ids


def phase2_inmaps(sizes, assign, slot_ids, x, gate, w1, b1, w2, b2, bf16):
    wdt = ml_dtypes.bfloat16 if bf16 else np.float32
    x_flat = x.reshape(B * S, D)
    flat_gate = gate.reshape(-1).astype(np.float32)
    C = sum(sizes)
    experts_used = sorted({e for row in assign for e in row})
    w1packed = {e: pack_w1(w1[e].astype(wdt)) for e in experts_used}
    w2packed = {e: pack_w2(w2[e].astype(wdt)) for e in experts_used}
    b1resh = {e: np.ascontiguousarray(b1[e].reshape(NF, 128).T) for e in experts_used}
    b2resh = {e: np.ascontiguousarray(b2[e].reshape(ND, 128).T) for e in experts_used}
    in_maps = []
    for core in range(NCORES):
        xg = np.zeros((C, D), np.float32)
        gt = np.zeros((C,), np.float32)
        m = {}
        o_base = 0
        for j, sj in enumerate(sizes):
            e = assign[core][j]
            ids = slot_ids[core][j]
            n = len(ids)
            if n:
                xg[o_base : o_base + n] = x_flat[ids]
                gt[o_base : o_base + n] = flat_gate[ids]
            m[f"w1p{j}"] = w1packed[e]
            m[f"b1r{j}"] = b1resh[e]
            m[f"w2p{j}"] = w2packed[e]
            m[f"b2r{j}"] = b2resh[e]
            o_base += sj
        m["xgT"] = np.ascontiguousarray(xg.T.astype(wdt))
        m["gateb"] = np.broadcast_to(gt[None, :], (128, C)).copy()
        in_maps.append(m)
    return in_maps


def kernel(x, router_w, w1, b1, w2, b2):
    prec = DEFAULT_PREC
    x = np.asarray(x, np.float32)
    router_w = np.asarray(router_w, np.float32)
    w1 = np.asarray(w1, np.float32)
    b1 = np.asarray(b1, np.float32)
    w2 = np.asarray(w2, np.float32)
    b2 = np.asarray(b2, np.float32)

    idx, gate = run_phase1(x, router_w)  # [B, S] each

    sizes, assign, slot_ids = phase2_plan(idx)
    bf16 = prec == "bf16"
    nc2 = build_phase2_seg(sizes, bf16=bf16)
    in_maps = phase2_inmaps(sizes, assign, slot_ids, x, gate, w1, b1, w2, b2, bf16)
    res2 = _run_spmd(nc2, in_maps, list(range(NCORES)))

    y_flat = np.zeros((B * S, D), np.float32)
    for core in range(NCORES):
        ygT = res2.results[core]["ygT"]  # [D, C]
        o_base = 0
        for j, sj in enumerate(sizes):
            ids = slot_ids[core][j]
            n = len(ids)
            if n:
                y_flat[ids] = ygT[:, o_base : o_base + n].T
            o_base += sj
    return y_flat.reshape(B, S, D)
